# revision 1
# baseline (speedup 1.0000x reference)
"""Multi-head causal self-attention (N=4, L=2048, E=1024, H=16) on 8 NeuronCores.

Sharding: core c handles batch b = c//2 and head-group g = c%2 (8 heads,
E-slice of 512). Each core computes its QKV projection slice, causal
attention for its 8 heads, and a partial out-projection (E-contraction over
its 512-slice). Host sums the two partials per batch (bias added on g=0).

On-chip layout (per core):
  qT/kT: [e_out(512) x L] transposed activations (4 tiles of [128, 2048])
  v:     [L x e_out] natural layout, per l-block tiles [128, 8 heads, 65]
         (65th column = 1.0 -> the ones column makes the attention matmul
          also produce the softmax denominator as output row 64)
  scores are computed transposed: s^T[l_k, l_q] = k^T.T @ q^T, so the
  av matmul (lhsT = v tile, rhs = exp(s^T)) needs no transposes at all.
  Softmax uses no max-subtraction (scores*scale is O(1) by construction),
  masking is multiplicative post-exp on the block-diagonal tiles only.
All matmuls run as float32r (single-pass, 1 cyc/row at N>=256).

Scheduling: phase A projects k and v. Phase B alternates q-projection for
one 512-wide l_q block with the attention over that block, so projection
matmuls fill the PE while attention is ACT(exp)-bound, keeping the PE busy
enough that the HAM clock gate stays at full rate. Within attention the
av matmuls trail their scores group by one group and the normalize /
out-projection chains trail by one head (deferred emission queue), so the
in-order PE queue never waits on ACT or DVE results.
"""

from collections import deque
from contextlib import ExitStack

import numpy as np

import concourse.bacc as bacc
import concourse.mybir as mybir
import concourse.tile as tile
from concourse import bass_utils

F32 = mybir.dt.float32
F32R = mybir.dt.float32r
AF = mybir.ActivationFunctionType

N, L, E = 4, 2048, 1024
H, EH = 16, 64
NCORES = 8
ES = 512          # e-slice per core (8 heads x 64)
SCALE = 1.0 / np.sqrt(EH)

_CACHE = {}


def _build():
    nc = bacc.Bacc("TRN2", target_bir_lowering=False, debug=False,
                   num_devices=NCORES)
    xq = nc.dram_tensor("xq", (E, L), F32, kind="ExternalInput").ap()
    xk = nc.dram_tensor("xk", (E, L), F32, kind="ExternalInput").ap()
    xv = nc.dram_tensor("xv", (E, L), F32, kind="ExternalInput").ap()
    wq = nc.dram_tensor("wq", (E, ES), F32, kind="ExternalInput").ap()
    wk = nc.dram_tensor("wk", (E, ES), F32, kind="ExternalInput").ap()
    wv = nc.dram_tensor("wv", (E, ES), F32, kind="ExternalInput").ap()
    wo = nc.dram_tensor("wo", (ES, E), F32, kind="ExternalInput").ap()
    bq = nc.dram_tensor("bq", (128, 4), F32, kind="ExternalInput").ap()
    bk = nc.dram_tensor("bk", (128, 4), F32, kind="ExternalInput").ap()
    bv = nc.dram_tensor("bv", (1, ES), F32, kind="ExternalInput").ap()
    bo = nc.dram_tensor("bo", (1, E), F32, kind="ExternalInput").ap()
    msk = nc.dram_tensor("msk", (4, 128, 512), F32, kind="ExternalInput").ap()
    y = nc.dram_tensor("y", (L, E), F32, kind="ExternalOutput").ap()

    with tile.TileContext(nc) as tc:
        with tc.tile_pool(name="const", bufs=1) as cpool, \
             tc.tile_pool(name="kt", bufs=4) as ktpool, \
             tc.tile_pool(name="vp", bufs=16) as vpool, \
             tc.tile_pool(name="wo", bufs=4) as wopool, \
             tc.tile_pool(name="wtq", bufs=8) as wqpool:

            kt = [ktpool.tile([128, L], F32R, tag="kt", name=f"kt{i}")
                  for i in range(4)]
            vts = [vpool.tile([128, 8, 65], F32R, tag="v", name=f"v{i}")
                   for i in range(16)]

            # ---------------- Phase A: k and v projections ----------------
            with tc.tile_pool(name="wtile", bufs=16) as wpool, \
                 tc.tile_pool(name="xs", bufs=2) as xpool, \
                 tc.tile_pool(name="ps1", bufs=5, space="PSUM") as ps1:

                def load_w(w_dram, nm):
                    ts = []
                    for ko in range(8):
                        t = wpool.tile([128, ES], F32R, tag=f"w{nm}",
                                       name=f"w{nm}{ko}")
                        nc.sync.dma_start(
                            out=t,
                            in_=w_dram[ko * 128:(ko + 1) * 128, :]
                            .bitcast(F32R))
                        ts.append(t)
                    return ts

                def x_chunk(x_dram, lb):
                    xt = xpool.tile([128, 8, 512], F32R, tag="x", name="xt")
                    nc.sync.dma_start(
                        out=xt,
                        in_=x_dram.rearrange("(ko ki) l -> ki ko l", ki=128)
                        [:, :, lb * 512:(lb + 1) * 512].bitcast(F32R))
                    return xt

                xk_0 = x_chunk(xk, 0)
                wk_t = load_w(wk, "k")
                xv_0 = x_chunk(xv, 0)
                wv_t = load_w(wv, "v")

                ones_st = cpool.tile([1, 128], F32)
                nc.vector.memset(ones_st, 1.0)
                ones = cpool.tile([1, 128], F32R)
                nc.vector.tensor_copy(ones, ones_st)
                vcol = cpool.tile([128, 8], F32)
                nc.vector.memset(vcol, 1.0)
                for lv in range(16):
                    nc.vector.tensor_copy(vts[lv][:, :, 64], vcol)
                bk_sb = cpool.tile([128, 4], F32)
                bv_sb = cpool.tile([1, ES], F32R)
                nc.sync.dma_start(out=bk_sb, in_=bk)
                nc.sync.dma_start(out=bv_sb, in_=bv.bitcast(F32R))

                wq_t = []
                for ko in range(8):
                    t = wqpool.tile([128, ES], F32R, tag="wq", name=f"wq{ko}")
                    nc.sync.dma_start(
                        out=t,
                        in_=wq[ko * 128:(ko + 1) * 128, :].bitcast(F32R))
                    wq_t.append(t)
                wo_t = []
                for pr in range(4):
                    t = wopool.tile([128, E], F32R, tag="wo", name=f"wo{pr}")
                    nc.sync.dma_start(
                        out=t,
                        in_=wo[pr * 128:(pr + 1) * 128, :].bitcast(F32R))
                    wo_t.append(t)

                for lb in range(4):
                    xk_t = xk_0 if lb == 0 else x_chunk(xk, lb)
                    xv_t = xv_0 if lb == 0 else x_chunk(xv, lb)
                    for eo in range(4):
                        ps = ps1.tile([128, 512], F32, tag="ps1", name="ps")
                        for ko in range(8):
                            nc.tensor.matmul(
                                ps, wk_t[ko][:, eo * 128:(eo + 1) * 128],
                                xk_t[:, ko, :],
                                start=(ko == 0), stop=(ko == 7))
                        nc.scalar.activation(
                            kt[eo][:, lb * 512:(lb + 1) * 512],
                            ps, AF.Identity,
                            bias=bk_sb[:, eo:eo + 1], scale=1.0)
                    for i in range(4):
                        lv = lb * 4 + i
                        ps = ps1.tile([128, 512], F32, tag="ps1", name="ps")
                        for ko in range(8):
                            nc.tensor.matmul(
                                ps, xv_t[:, ko, i * 128:(i + 1) * 128],
                                wv_t[ko], start=(ko == 0), stop=False)
                        nc.tensor.matmul(ps, ones, bv_sb,
                                         start=False, stop=True)
                        nc.scalar.copy(
                            vts[lv][:, :, 0:64],
                            ps[:, :].rearrange("p (h e) -> p h e", e=64))

            # -------- Phase B: q projection interleaved with attention -----
            with ExitStack() as stk:
                qjpool = stk.enter_context(tc.tile_pool(name="qj", bufs=8))
                xqpool = stk.enter_context(tc.tile_pool(name="xqp", bufs=1))
                sp = stk.enter_context(tc.tile_pool(name="sp", bufs=2, space="PSUM"))
                op = stk.enter_context(tc.tile_pool(name="op", bufs=3, space="PSUM"))
                fp = stk.enter_context(tc.tile_pool(name="fp", bufs=1, space="PSUM"))
                ppool = stk.enter_context(tc.tile_pool(name="pp", bufs=3))
                rpool = stk.enter_context(tc.tile_pool(name="rp", bufs=1))
                rbpool = stk.enter_context(tc.tile_pool(name="rb", bufs=2))
                nmpool = stk.enter_context(tc.tile_pool(name="nm", bufs=2))
                a2pool = stk.enter_context(tc.tile_pool(name="a2", bufs=8))
                otpool = stk.enter_context(tc.tile_pool(name="ot", bufs=2))

                bq_sb = cpool.tile([128, 4], F32)
                bo_sb = cpool.tile([1, E], F32R)
                mask_sb = cpool.tile([128, 4, 512], F32)
                nc.sync.dma_start(out=bq_sb, in_=bq)
                nc.sync.dma_start(out=bo_sb, in_=bo.bitcast(F32R))
                nc.sync.dma_start(out=mask_sb,
                                  in_=msk.rearrange("m p q -> p m q"))

                pending = deque()

                def flush_one():
                    while len(pending) > 3:
                        pending.popleft()()

                def flush_all():
                    while pending:
                        pending.popleft()()

                def mk_av(pso, h, g, nkb, p2):
                    def emit():
                        for i in range(2):
                            kb = 2 * g + i
                            nc.tensor.matmul(
                                pso, vts[kb][:, h, :], p2[:, i, :],
                                start=(kb == 0), stop=(kb == nkb - 1))
                    return emit

                def mk_norm_dve(pso, cell):
                    def emit():
                        dsb = rpool.tile([1, 512], F32, tag="dsb", name="dsb")
                        nc.vector.tensor_copy(dsb, pso[64:65, :])
                        rcf = rpool.tile([1, 512], F32, tag="rcf", name="rcf")
                        nc.vector.reciprocal_approx_fast(rcf, dsb)
                        rc = rpool.tile([1, 512], F32R, tag="rc", name="rc")
                        nc.vector.tensor_copy(rc, rcf)
                        cell.append(rc)
                    return emit

                def mk_norm_mm(pso, cell, at2, t, po):
                    def emit():
                        rc = cell.pop()
                        psb = fp.tile([64, 512], F32, tag="fp", name="psb")
                        nc.tensor.matmul(psb, ones[:, 0:64], rc,
                                         start=True, stop=True)
                        rb = rbpool.tile([64, 512], F32R, tag="rb", name="rb")
                        nc.vector.tensor_copy(rb, psb)
                        if po == 0:
                            nc.vector.tensor_mul(at2[t][0:64, :],
                                                 pso[0:64, :], rb)
                        else:
                            nrm = nmpool.tile([64, 512], F32R, tag="nrm",
                                              name="nrm")
                            nc.vector.tensor_mul(nrm, pso[0:64, :], rb)
                            # partition shift 0-63 -> 64-127 via sbuf-sbuf DMA
                            nc.sync.dma_start(out=at2[t][64:128, :], in_=nrm)
                    return emit

                def mk_outproj_parts(at2, jq):
                    def mk_one(lc, no):
                        def emit():
                            psf = fp.tile([128, 512], F32, tag="fp",
                                          name="psf")
                            for pr in range(4):
                                nc.tensor.matmul(
                                    psf,
                                    at2[pr][:, lc * 128:(lc + 1) * 128],
                                    wo_t[pr][:, no * 512:(no + 1) * 512],
                                    start=(pr == 0), stop=False)
                            nc.tensor.matmul(
                                psf, ones,
                                bo_sb[:, no * 512:(no + 1) * 512],
                                start=False, stop=True)
                            ot = otpool.tile([128, 512], F32, tag="ot",
                                             name="ot")
                            nc.vector.tensor_copy(ot, psf)
                            nc.sync.dma_start(
                                out=y[jq * 512 + lc * 128:
                                      jq * 512 + (lc + 1) * 128,
                                      no * 512:(no + 1) * 512],
                                in_=ot)
                        return emit
                    return [mk_one(lc, no)
                            for lc in range(4) for no in range(2)]

                for jq in range(4):
                    # q projection for this l_q block (psum via the sp pool)
                    xq_t = xqpool.tile([128, 8, 512], F32R, tag="xq",
                                       name="xqt")
                    nc.gpsimd.dma_start(
                        out=xq_t,
                        in_=xq.rearrange("(ko ki) l -> ki ko l", ki=128)
                        [:, :, jq * 512:(jq + 1) * 512].bitcast(F32R))
                    qj = [qjpool.tile([128, 512], F32R, tag="qj",
                                      name=f"qj{i}") for i in range(4)]
                    for ep in range(2):           # eo pairs
                        psq = sp.tile([128, 2, 512], F32, tag="sp",
                                      name="psq")
                        for ei in range(2):
                            eo = 2 * ep + ei
                            for ko in range(8):
                                nc.tensor.matmul(
                                    psq[:, ei, :],
                                    wq_t[ko][:, eo * 128:(eo + 1) * 128],
                                    xq_t[:, ko, :],
                                    start=(ko == 0), stop=(ko == 7))
                            flush_one()
                        for ei in range(2):
                            eo = 2 * ep + ei
                            nc.scalar.activation(
                                qj[eo], psq[:, ei, :], AF.Identity,
                                bias=bq_sb[:, eo:eo + 1], scale=1.0)

                    at2 = [a2pool.tile([128, 512], F32R, tag="a2",
                                       name=f"a2_{i}") for i in range(4)]
                    nkb = 4 * (jq + 1)
                    for h in range(8):
                        t, po = h // 2, (h % 2) * 64
                        pso = op.tile([65, 512], F32, tag="op", name="pso")
                        for g in range(2 * (jq + 1)):
                            pss = sp.tile([128, 2, 512], F32, tag="sp",
                                          name="pss")
                            for i in range(2):
                                kb = 2 * g + i
                                nc.tensor.matmul(
                                    pss[:, i, :],
                                    kt[t][po:po + 64,
                                          kb * 128:(kb + 1) * 128],
                                    qj[t][po:po + 64, :],
                                    start=True, stop=True)
                            p2 = ppool.tile([128, 2, 512], F32R, tag="p",
                                            name="p2")
                            nc.scalar.activation(p2, pss, AF.Exp,
                                                 scale=float(SCALE))
                            if g >= 2 * jq:   # block-diagonal: needs mask
                                mi = 2 * (g - 2 * jq)
                                nc.vector.tensor_mul(
                                    p2, p2, mask_sb[:, mi:mi + 2, :])
                            flush_one()
                            pending.append(mk_av(pso, h, g, nkb, p2))
                        cell = []
                        pending.append(mk_norm_dve(pso, cell))
                        pending.append(mk_norm_mm(pso, cell, at2, t, po))
                    for part in mk_outproj_parts(at2, jq):
                        pending.append(part)
                flush_all()

    nc.finalize()
    return nc


def _make_masks():
    kk = np.arange(128)[:, None]
    qq = np.arange(512)[None, :]
    return np.stack([(qq >= kk + 128 * m) for m in range(4)]
                    ).astype(np.float32)


def make_in_maps(query, key, value, W_packed, b_packed, W_out, b_out):
    query = np.asarray(query, dtype=np.float32)
    key = np.asarray(key, dtype=np.float32)
    value = np.asarray(value, dtype=np.float32)
    W_packed = np.asarray(W_packed, dtype=np.float32)
    b_packed = np.asarray(b_packed, dtype=np.float32)
    W_out = np.asarray(W_out, dtype=np.float32)
    b_out = np.asarray(b_out, dtype=np.float32)

    msk = _make_masks()
    xqT = [np.ascontiguousarray(query[b].T) for b in range(N)]
    xkT = [np.ascontiguousarray(key[b].T) for b in range(N)]
    xvT = [np.ascontiguousarray(value[b].T) for b in range(N)]

    in_maps = []
    for c in range(NCORES):
        b, g = c // 2, c % 2
        sl = slice(g * ES, (g + 1) * ES)
        in_maps.append({
            "xq": xqT[b], "xk": xkT[b], "xv": xvT[b],
            "wq": np.ascontiguousarray(W_packed[0 * E:][:E][sl, :].T),
            "wk": np.ascontiguousarray(W_packed[1 * E:][:E][sl, :].T),
            "wv": np.ascontiguousarray(W_packed[2 * E:][:E][sl, :].T),
            "wo": np.ascontiguousarray(W_out[:, sl].T),
            "bq": np.ascontiguousarray(
                b_packed[0 * E:][:E][sl].reshape(4, 128).T),
            "bk": np.ascontiguousarray(
                b_packed[1 * E:][:E][sl].reshape(4, 128).T),
            "bv": b_packed[2 * E:][:E][sl].reshape(1, ES).copy(),
            "bo": (b_out.reshape(1, E).copy() if g == 0
                   else np.zeros((1, E), np.float32)),
            "msk": msk,
        })
    return in_maps


def get_nc():
    if "nc" not in _CACHE:
        _CACHE["nc"] = _build()
    return _CACHE["nc"]


def kernel(query, key, value, W_packed, b_packed, W_out, b_out):
    nc = get_nc()
    in_maps = make_in_maps(query, key, value, W_packed, b_packed,
                           W_out, b_out)
    res = bass_utils.run_bass_kernel_spmd(nc, in_maps,
                                          core_ids=list(range(NCORES)))
    out = np.stack([res.results[2 * b]["y"] + res.results[2 * b + 1]["y"]
                    for b in range(N)])
    return out.astype(np.float32)



# revision 29
# speedup vs baseline: 1.4978x; 1.4978x over previous
"""Multi-head causal self-attention (N=4, L=2048, E=1024, H=16) on 8 NeuronCores.

Sharding: core c handles batch b = c//2 and head-group g = c%2 (8 heads,
E-slice of 512). Each core computes its QKV projection slice, causal
attention for its 8 heads, and a partial out-projection (E-contraction over
its 512-slice). Host sums the two partials per batch (bias added on g=0).

Key structure (v2, rewritten from the 533us baseline):
  - Scores for the two heads of a kt/qj tile run as ROW-TILED CONCURRENT
    matmul pairs (K=64 each, rows 0-63 / 64-127) -> one 512-cycle slot
    covers both heads, and each pair's LDWEIGHTS pulls ahead under the
    other's matmul (disjoint row groups).
  - Diagonal 128-blocks stream only columns >= 128*m (rest is causally
    masked): shorter score/AV matmuls and shorter exps.
  - kt/qj/p2/vts are bf16 (tolerance 2e-2 vs measured ~2e-4 error budget);
    exp output is bf16, mask-mul on DVE gets 2x mode, AV matmul is bf16.
  - v tiles carry a ones column (65th) so the AV matmul also produces the
    softmax denominator (row 64 of the [65,512] psum).
  - All bias adds that needed K=1 matmuls now ride on DVE/GPSIMD adds with
    pre-broadcast bias tiles; q/k biases stay on ACT (per-partition bias).
  - Projection work for l-blocks 1..3 and q-blocks 1..3 is deferred and
    interleaved into the attention unit stream as PE filler (deadline
    scheduled) so the PE never starves while ACT runs exp -> HAM stays at
    K=8/8.
"""

from collections import deque
from contextlib import ExitStack

import ml_dtypes
import numpy as np

import concourse.bacc as bacc
import concourse.mybir as mybir
import concourse.tile as tile
from concourse import bass_utils

F32 = mybir.dt.float32
F32R = mybir.dt.float32r
BF16 = mybir.dt.bfloat16
AF = mybir.ActivationFunctionType

N, L, E = 4, 2048, 1024
H, EH = 16, 64
NCORES = 8
ES = 512          # e-slice per core (8 heads x 64)
SCALE = 1.0 / np.sqrt(EH)

_CACHE = {}


def _build():
    nc = bacc.Bacc("TRN2", target_bir_lowering=False, debug=False,
                   num_devices=NCORES)
    xq = nc.dram_tensor("xq", (E, L), BF16, kind="ExternalInput").ap()
    xk = nc.dram_tensor("xk", (E, L), BF16, kind="ExternalInput").ap()
    xv = nc.dram_tensor("xv", (E, L), BF16, kind="ExternalInput").ap()
    wq = nc.dram_tensor("wq", (E, ES), BF16, kind="ExternalInput").ap()
    wk = nc.dram_tensor("wk", (E, ES), BF16, kind="ExternalInput").ap()
    wv = nc.dram_tensor("wv", (E, ES), BF16, kind="ExternalInput").ap()
    wo = nc.dram_tensor("wo", (ES, E), BF16, kind="ExternalInput").ap()
    bq = nc.dram_tensor("bq", (128, 4), F32, kind="ExternalInput").ap()
    bk = nc.dram_tensor("bk", (128, 4), F32, kind="ExternalInput").ap()
    bv = nc.dram_tensor("bv", (1, ES), F32, kind="ExternalInput").ap()
    bo = nc.dram_tensor("bo", (1, E), F32, kind="ExternalInput").ap()
    msk = nc.dram_tensor("msk", (128, 128), F32, kind="ExternalInput").ap()
    y = nc.dram_tensor("y", (L, E), F32, kind="ExternalOutput").ap()

    with tile.TileContext(nc) as tc:
        with ExitStack() as stk:
            ec = stk.enter_context
            cpool = ec(tc.tile_pool(name="const", bufs=1))
            ktpool = ec(tc.tile_pool(name="kt", bufs=4))
            vpool = ec(tc.tile_pool(name="vp", bufs=16))
            wkpool = ec(tc.tile_pool(name="wtk", bufs=8))
            wvpool = ec(tc.tile_pool(name="wtv", bufs=8))
            wqpool = ec(tc.tile_pool(name="wtq", bufs=8))
            wopool = ec(tc.tile_pool(name="wo", bufs=4))
            xpool = ec(tc.tile_pool(name="xs", bufs=2))
            qjpool = ec(tc.tile_pool(name="qj", bufs=8))
            a2pool = ec(tc.tile_pool(name="a2", bufs=16))
            ppool = ec(tc.tile_pool(name="pp", bufs=5))
            rpool = ec(tc.tile_pool(name="rp", bufs=2))
            rbpool = ec(tc.tile_pool(name="rb", bufs=2))
            nmpool = ec(tc.tile_pool(name="nm", bufs=2))
            otpool = ec(tc.tile_pool(name="ot", bufs=3))
            sp = ec(tc.tile_pool(name="sp", bufs=2, space="PSUM"))
            op = ec(tc.tile_pool(name="op", bufs=3, space="PSUM"))
            fp = ec(tc.tile_pool(name="fp", bufs=1, space="PSUM"))

            # ---------------- DMA issue: weights & consts ----------------
            # sync queue: wk, xk0, wv, xv0, xq0  (critical path to first MMs)
            # gpsimd queue: small consts, wq, wo (parallel queue)
            bq_sb = cpool.tile([128, 4], F32)
            bk_sb = cpool.tile([128, 4], F32)
            bv_sb = cpool.tile([1, ES], F32R)
            bo_sb = cpool.tile([1, E], F32R)
            mtri_f = cpool.tile([128, 128], F32)
            nc.gpsimd.dma_start(out=bq_sb, in_=bq)
            nc.gpsimd.dma_start(out=bk_sb, in_=bk)
            nc.gpsimd.dma_start(out=bv_sb, in_=bv.bitcast(F32R))
            nc.gpsimd.dma_start(out=bo_sb, in_=bo.bitcast(F32R))
            nc.gpsimd.dma_start(out=mtri_f, in_=msk)

            def load_w(pool, w_dram, nm, eng):
                ts = []
                for ko in range(8):
                    t = pool.tile([128, ES], BF16, tag=f"w{nm}",
                                  name=f"w{nm}{ko}")
                    eng.dma_start(
                        out=t,
                        in_=w_dram[ko * 128:(ko + 1) * 128, :])
                    ts.append(t)
                return ts

            wk_t = load_w(wkpool, wk, "k", nc.sync)
            wv_t = load_w(wvpool, wv, "v", nc.gpsimd)
            wq_t = load_w(wqpool, wq, "q", nc.gpsimd)
            wo_t = []
            for pr in range(4):
                t = wopool.tile([128, E], BF16, tag="wo", name=f"wo{pr}")
                nc.gpsimd.dma_start(
                    out=t, in_=wo[pr * 128:(pr + 1) * 128, :])
                wo_t.append(t)

            # ---------------- persistent on-chip tensors -----------------
            kt = [ktpool.tile([128, L], BF16, tag="kt", name=f"kt{i}")
                  for i in range(4)]
            vts = [vpool.tile([128, 8, 66], BF16, tag="v", name=f"v{i}")
                   for i in range(16)]

            ones_st = cpool.tile([1, 128], F32)
            nc.vector.memset(ones_st, 1.0)
            ones = cpool.tile([1, 128], F32R)
            nc.vector.tensor_copy(ones, ones_st)
            for lv in range(16):
                nc.vector.memset(vts[lv][:, :, 64], 1.0)
            mtri = cpool.tile([128, 2, 128], BF16)
            nc.vector.tensor_copy(mtri[:, 0, :], mtri_f)
            nc.vector.tensor_copy(mtri[:, 1, :], mtri_f)

            # broadcast bias tiles (via ones-matmul, one-time)
            bv_bc = cpool.tile([128, ES], F32)
            bo_bc = cpool.tile([128, E], F32)

            def make_bcasts():
                ps = fp.tile([128, 512], F32, tag="fp", name="psb0")
                nc.tensor.matmul(ps, ones, bv_sb, start=True, stop=True)
                nc.vector.tensor_copy(bv_bc, ps)
                for half in range(2):
                    ps2 = fp.tile([128, 512], F32, tag="fp", name="psb1")
                    nc.tensor.matmul(ps2, ones,
                                     bo_sb[:, half * 512:(half + 1) * 512],
                                     start=True, stop=True)
                    nc.vector.tensor_copy(
                        bo_bc[:, half * 512:(half + 1) * 512], ps2)

            # ---------------- projection chunk emitters ------------------
            x_cache = {}

            def x_tile(x_dram, lb, key, eng=None):
                # NOTE: xpool ring has bufs=2; allocation order must ensure
                # the slot being reused already has its readers emitted.
                if (key, lb) in x_cache:
                    return x_cache[(key, lb)]
                t = xpool.tile([128, 8, 512], BF16, tag="x", name="xt")
                (eng or nc.sync).dma_start(
                    out=t,
                    in_=x_dram.rearrange("(ko ki) l -> ki ko l", ki=128)
                    [:, :, lb * 512:(lb + 1) * 512])
                x_cache[(key, lb)] = t
                return t

            def kproj_chunk(lb, ep):
                # kt[eo][:, lb*512:+512] for eo in {2ep, 2ep+1}
                def emit():
                    xt = x_tile(xk, lb, "k")
                    ps = sp.tile([128, 2, 512], F32, tag="sp", name="psk")
                    for ei in range(2):
                        eo = 2 * ep + ei
                        for ko in range(8):
                            nc.tensor.matmul(
                                ps[:, ei, :],
                                wk_t[ko][:, eo * 128:(eo + 1) * 128],
                                xt[:, ko, :],
                                start=(ko == 0), stop=(ko == 7))
                    for ei in range(2):
                        eo = 2 * ep + ei
                        nc.vector.tensor_scalar_add(
                            kt[eo][:, lb * 512:(lb + 1) * 512],
                            ps[:, ei, :], bk_sb[:, eo:eo + 1])
                return emit

            def vproj_chunk(lb, pair):
                # vts[lb*4 + {2pair, 2pair+1}]
                def emit():
                    xt = x_tile(xv, lb, "v")
                    ps = sp.tile([128, 2, 512], F32, tag="sp", name="psv")
                    for ii in range(2):
                        i = 2 * pair + ii
                        for ko in range(8):
                            nc.tensor.matmul(
                                ps[:, ii, :],
                                xt[:, ko, i * 128:(i + 1) * 128],
                                wv_t[ko], start=(ko == 0), stop=(ko == 7))
                    for ii in range(2):
                        i = 2 * pair + ii
                        lv = lb * 4 + i
                        nc.vector.tensor_add(
                            vts[lv][:, :, 0:64],
                            ps[:, ii, :].rearrange("p (h e) -> p h e", e=64),
                            bv_bc.rearrange("p (h e) -> p h e", e=64))
                return emit

            qj = {}

            def qproj_chunk(jq, ep):
                # qj[(jq, eo)] for eo in {2ep, 2ep+1}
                def emit():
                    xt = x_tile(xq, jq, "q")
                    ps = sp.tile([128, 2, 512], F32, tag="sp", name="psq")
                    for ei in range(2):
                        eo = 2 * ep + ei
                        for ko in range(8):
                            nc.tensor.matmul(
                                ps[:, ei, :],
                                wq_t[ko][:, eo * 128:(eo + 1) * 128],
                                xt[:, ko, :],
                                start=(ko == 0), stop=(ko == 7))
                    for ei in range(2):
                        eo = 2 * ep + ei
                        t = qjpool.tile([128, 512], BF16, tag="qj",
                                        name=f"qj{jq}_{eo}")
                        nc.vector.tensor_scalar_add(
                            t, ps[:, ei, :], bq_sb[:, eo:eo + 1])
                        qj[(jq, eo)] = t
                return emit

            at2 = {}

            def outproj_part(jq, lc, no):
                def emit():
                    a = at2[jq]
                    psf = fp.tile([128, 512], F32, tag="fp", name="psf")
                    for pr in range(4):
                        nc.tensor.matmul(
                            psf,
                            a[pr][:, lc * 128:(lc + 1) * 128],
                            wo_t[pr][:, no * 512:(no + 1) * 512],
                            start=(pr == 0), stop=(pr == 3))
                    ot = otpool.tile([128, 512], F32, tag="ot", name="ot")
                    nc.vector.tensor_add(
                        ot, psf, bo_bc[:, no * 512:(no + 1) * 512])
                    nc.sync.dma_start(
                        out=y[jq * 512 + lc * 128:jq * 512 + (lc + 1) * 128,
                              no * 512:(no + 1) * 512],
                        in_=ot)
                return emit

            # ---------------- attention unit machinery -------------------
            pending = deque()

            def flush_pending(depth):
                while len(pending) > depth:
                    pending.popleft()()

            def mk_av_pair(psoA, psoB, t, kb, n0, nkb, p2):
                def emit():
                    nc.tensor.matmul(
                        psoA[:, n0:512], vts[kb][:, 2 * t, 0:65],
                        p2[:, 0, n0:512],
                        start=(kb == 0), stop=(kb == nkb - 1))
                    nc.tensor.matmul(
                        psoB[:, n0:512], vts[kb][:, 2 * t + 1, 0:65],
                        p2[:, 1, n0:512],
                        start=(kb == 0), stop=(kb == nkb - 1))
                return emit

            def mk_norm(psoA, psoB, t, jq):
                def emit():
                    # reciprocal of denominators (row 64), then broadcast
                    # via K=1 matmul, then normalize into at2 tiles.
                    # reciprocal_approx_fast mis-reads large values straight
                    # from PSUM (negative garbage); stage through SBUF first
                    dsbA = rpool.tile([1, 512], F32, tag="dsb", name="dsbA")
                    nc.vector.tensor_copy(dsbA, psoA[64:65, :])
                    rcf = rpool.tile([1, 512], F32, tag="rcf", name="rcf")
                    nc.vector.reciprocal_approx_fast(rcf, dsbA)
                    rcA = rpool.tile([1, 512], F32R, tag="rc", name="rcA")
                    nc.vector.tensor_copy(rcA, rcf)
                    dsbB = rpool.tile([1, 512], F32, tag="dsb", name="dsbB")
                    nc.vector.tensor_copy(dsbB, psoB[64:65, :])
                    rcg = rpool.tile([1, 512], F32, tag="rcf", name="rcg")
                    nc.vector.reciprocal_approx_fast(rcg, dsbB)
                    rcB = rpool.tile([1, 512], F32R, tag="rc", name="rcB")
                    nc.vector.tensor_copy(rcB, rcg)
                    psbA = fp.tile([64, 512], F32, tag="fp", name="psbA")
                    nc.tensor.matmul(psbA, ones[:, 0:64], rcA,
                                     start=True, stop=True)
                    psbB = fp.tile([64, 512], F32, tag="fp", name="psbB")
                    nc.tensor.matmul(psbB, ones[:, 0:64], rcB,
                                     start=True, stop=True)
                    rbA = rbpool.tile([64, 512], F32R, tag="rb", name="rbA")
                    nc.vector.tensor_copy(rbA, psbA)
                    rbB = rbpool.tile([64, 512], F32R, tag="rb", name="rbB")
                    nc.vector.tensor_copy(rbB, psbB)
                    nc.vector.tensor_mul(at2[jq][t][0:64, :],
                                         psoA[0:64, :], rbA)
                    nrm = nmpool.tile([64, 512], BF16, tag="nrm", name="nrm")
                    nc.vector.tensor_mul(nrm, psoB[0:64, :], rbB)
                    # partition shift 0-63 -> 64-127 via sbuf-sbuf DMA
                    nc.sync.dma_start(out=at2[jq][t][64:128, :], in_=nrm)
                return emit

            def unit(jq, t, kb):
                nkb = 4 * (jq + 1)
                m = kb - 4 * jq
                n0 = 128 * m if m >= 0 else 0
                pss = sp.tile([128, 2, 512], F32, tag="sp", name="pss")
                nc.tensor.matmul(
                    pss[:, 0, n0:512],
                    kt[t][0:64, kb * 128:(kb + 1) * 128],
                    qj[(jq, t)][0:64, n0:512], start=True, stop=True)
                nc.tensor.matmul(
                    pss[:, 1, n0:512],
                    kt[t][64:128, kb * 128:(kb + 1) * 128],
                    qj[(jq, t)][64:128, n0:512], start=True, stop=True)
                flush_pending(4)
                p2 = ppool.tile([128, 2, 512], BF16, tag="p", name="p2")
                nc.scalar.activation(p2[:, :, n0:512], pss[:, :, n0:512],
                                     AF.Exp, scale=float(SCALE))
                if m >= 0:
                    # all-SBUF operands: run on the idle gpsimd engine
                    nc.gpsimd.tensor_mul(
                        p2[:, :, n0:n0 + 128], p2[:, :, n0:n0 + 128], mtri)
                return p2

            # ---------------- opening (phase A head) ---------------------
            # x ring (bufs=2) allocation order: xk0(A), xv0(B), xq0(A after
            # kproj emitted), then per-seg xk/xv prefetch + mid-seg xq kick.
            x_tile(xk, 0, "k")
            x_tile(xv, 0, "v")
            kproj_chunk(0, 0)()
            make_bcasts()
            kproj_chunk(0, 1)()
            x_tile(xq, 0, "q")
            vproj_chunk(0, 0)()
            vproj_chunk(0, 1)()
            qproj_chunk(0, 0)()
            qproj_chunk(0, 1)()

            # ---------------- filler schedules per segment ---------------
            def xq_kick(jq):
                # issue next xq DMA once kproj fillers (readers of the x
                # ring slot being reused) have been emitted
                def emit():
                    x_tile(xq, jq, "q")
                return emit

            # ---------------- main attention stream ----------------------
            # Run-level global schedule. jq=2 and jq=3 runs interleave so
            # the exp-heavy tail shares PE filler (outproj parts) instead
            # of starving once projection work is exhausted.
            for jq in range(4):
                at2[jq] = [a2pool.tile([128, 512], BF16, tag="a2",
                                       name=f"a2_{jq}_{i}")
                           for i in range(4)]

            def pre(lb):
                def em():
                    x_tile(xk, lb, "k")
                    x_tile(xv, lb, "v")
                return em

            op0 = [outproj_part(0, lc, no) for lc in range(4)
                   for no in range(2)]
            op1 = [outproj_part(1, lc, no) for lc in range(4)
                   for no in range(2)]
            op2 = [outproj_part(2, lc, no) for lc in range(4)
                   for no in range(2)]
            op3 = [outproj_part(3, lc, no) for lc in range(4)
                   for no in range(2)]

            schedule = [
                (0, 0, [pre(1), kproj_chunk(1, 0)]),
                (0, 1, [kproj_chunk(1, 1), xq_kick(1)]),
                (0, 2, [vproj_chunk(1, 0), vproj_chunk(1, 1)]),
                (0, 3, [qproj_chunk(1, 0), qproj_chunk(1, 1)]),
                (1, 0, [pre(2), kproj_chunk(2, 0), kproj_chunk(2, 1)]),
                (1, 1, [xq_kick(2), vproj_chunk(2, 0),
                        vproj_chunk(2, 1)]),
                (1, 2, op0[0:4]),
                (1, 3, op0[4:8] + [qproj_chunk(2, 0), qproj_chunk(2, 1)]),
                (2, 0, [pre(3), kproj_chunk(3, 0), kproj_chunk(3, 1),
                        xq_kick(3), vproj_chunk(3, 0), vproj_chunk(3, 1),
                        qproj_chunk(3, 0), qproj_chunk(3, 1)]),
                (3, 0, op1[0:3]),
                (2, 1, op1[3:6]),
                (3, 1, op1[6:8]),
                (2, 2, []),
                (3, 2, []),
                (2, 3, []),
                (3, 3, op2[0:8]),
            ]
            seen_jq = set()
            for jq, t, fills in schedule:
                if jq not in seen_jq:
                    # force pending (incl. this jq's qproj fillers) to have
                    # emitted before the first unit references qj[(jq, t)]
                    flush_pending(0)
                    seen_jq.add(jq)
                nkb = 4 * (jq + 1)
                psoA = op.tile([65, 512], F32, tag="op", name="psoA")
                psoB = op.tile([65, 512], F32, tag="op", name="psoB")
                done_fill = 0
                for kb in range(nkb):
                    p2 = unit(jq, t, kb)
                    m = kb - 4 * jq
                    n0 = 128 * m if m >= 0 else 0
                    pending.append(
                        mk_av_pair(psoA, psoB, t, kb, n0, nkb, p2))
                    want = ((kb + 1) * len(fills)) // nkb
                    while done_fill < want:
                        pending.append(fills[done_fill])
                        done_fill += 1
                pending.append(mk_norm(psoA, psoB, t, jq))
            for part in op3:
                pending.append(part)
            flush_pending(0)

    nc.finalize()
    return nc


def _make_tri():
    kk = np.arange(128)[:, None]
    jj = np.arange(128)[None, :]
    return (jj >= kk).astype(np.float32)


def make_in_maps(query, key, value, W_packed, b_packed, W_out, b_out):
    query = np.asarray(query, dtype=np.float32)
    key = np.asarray(key, dtype=np.float32)
    value = np.asarray(value, dtype=np.float32)
    W_packed = np.asarray(W_packed, dtype=np.float32)
    b_packed = np.asarray(b_packed, dtype=np.float32)
    W_out = np.asarray(W_out, dtype=np.float32)
    b_out = np.asarray(b_out, dtype=np.float32)

    msk = _make_tri()
    BF = ml_dtypes.bfloat16
    xqT = [np.ascontiguousarray(query[b].T).astype(BF) for b in range(N)]
    xkT = [np.ascontiguousarray(key[b].T).astype(BF) for b in range(N)]
    xvT = [np.ascontiguousarray(value[b].T).astype(BF) for b in range(N)]

    in_maps = []
    for c in range(NCORES):
        b, g = c // 2, c % 2
        sl = slice(g * ES, (g + 1) * ES)
        in_maps.append({
            "xq": xqT[b], "xk": xkT[b], "xv": xvT[b],
            "wq": np.ascontiguousarray(
                W_packed[0 * E:][:E][sl, :].T).astype(BF),
            "wk": np.ascontiguousarray(
                W_packed[1 * E:][:E][sl, :].T).astype(BF),
            "wv": np.ascontiguousarray(
                W_packed[2 * E:][:E][sl, :].T).astype(BF),
            "wo": np.ascontiguousarray(W_out[:, sl].T).astype(BF),
            "bq": np.ascontiguousarray(
                b_packed[0 * E:][:E][sl].reshape(4, 128).T),
            "bk": np.ascontiguousarray(
                b_packed[1 * E:][:E][sl].reshape(4, 128).T),
            "bv": b_packed[2 * E:][:E][sl].reshape(1, ES).copy(),
            "bo": (b_out.reshape(1, E).copy() if g == 0
                   else np.zeros((1, E), np.float32)),
            "msk": msk,
        })
    return in_maps


def get_nc():
    if "nc" not in _CACHE:
        _CACHE["nc"] = _build()
    return _CACHE["nc"]


def kernel(query, key, value, W_packed, b_packed, W_out, b_out):
    nc = get_nc()
    in_maps = make_in_maps(query, key, value, W_packed, b_packed,
                           W_out, b_out)
    res = bass_utils.run_bass_kernel_spmd(nc, in_maps,
                                          core_ids=list(range(NCORES)))
    out = np.stack([res.results[2 * b]["y"] + res.results[2 * b + 1]["y"]
                    for b in range(N)])
    return out.astype(np.float32)


# revision 30
# speedup vs baseline: 1.5589x; 1.0407x over previous
"""Multi-head causal self-attention (N=4, L=2048, E=1024, H=16) on 8 NeuronCores.

Sharding: core c handles batch b = c//2 and head-group g = c%2 (8 heads,
E-slice of 512). Each core computes its QKV projection slice, causal
attention for its 8 heads, and a partial out-projection (E-contraction over
its 512-slice). Host sums the two partials per batch (bias added on g=0).

Key structure (v2, rewritten from the 533us baseline):
  - Scores for the two heads of a kt/qj tile run as ROW-TILED CONCURRENT
    matmul pairs (K=64 each, rows 0-63 / 64-127) -> one 512-cycle slot
    covers both heads, and each pair's LDWEIGHTS pulls ahead under the
    other's matmul (disjoint row groups).
  - Diagonal 128-blocks stream only columns >= 128*m (rest is causally
    masked): shorter score/AV matmuls and shorter exps.
  - kt/qj/p2/vts are bf16 (tolerance 2e-2 vs measured ~2e-4 error budget);
    exp output is bf16, mask-mul on DVE gets 2x mode, AV matmul is bf16.
  - v tiles carry a ones column (65th) so the AV matmul also produces the
    softmax denominator (row 64 of the [65,512] psum).
  - All bias adds that needed K=1 matmuls now ride on DVE/GPSIMD adds with
    pre-broadcast bias tiles; q/k biases stay on ACT (per-partition bias).
  - Projection work for l-blocks 1..3 and q-blocks 1..3 is deferred and
    interleaved into the attention unit stream as PE filler (deadline
    scheduled) so the PE never starves while ACT runs exp -> HAM stays at
    K=8/8.
"""

from collections import deque
from contextlib import ExitStack

import ml_dtypes
import numpy as np

import concourse.bacc as bacc
import concourse.mybir as mybir
import concourse.tile as tile
from concourse import bass_utils

F32 = mybir.dt.float32
F32R = mybir.dt.float32r
BF16 = mybir.dt.bfloat16
AF = mybir.ActivationFunctionType

N, L, E = 4, 2048, 1024
H, EH = 16, 64
NCORES = 8
ES = 512          # e-slice per core (8 heads x 64)
SCALE = 1.0 / np.sqrt(EH)

_CACHE = {}


def _build():
    nc = bacc.Bacc("TRN2", target_bir_lowering=False, debug=False,
                   num_devices=NCORES)
    xq = nc.dram_tensor("xq", (E, L), BF16, kind="ExternalInput").ap()
    xk = nc.dram_tensor("xk", (E, L), BF16, kind="ExternalInput").ap()
    xv = nc.dram_tensor("xv", (E, L), BF16, kind="ExternalInput").ap()
    wq = nc.dram_tensor("wq", (E, ES), BF16, kind="ExternalInput").ap()
    wk = nc.dram_tensor("wk", (E, ES), BF16, kind="ExternalInput").ap()
    wv = nc.dram_tensor("wv", (E, ES), BF16, kind="ExternalInput").ap()
    wo = nc.dram_tensor("wo", (ES, E), BF16, kind="ExternalInput").ap()
    bq = nc.dram_tensor("bq", (128, 4), F32, kind="ExternalInput").ap()
    bk = nc.dram_tensor("bk", (128, 4), F32, kind="ExternalInput").ap()
    bv = nc.dram_tensor("bv", (1, ES), F32, kind="ExternalInput").ap()
    bo = nc.dram_tensor("bo", (1, E), F32, kind="ExternalInput").ap()
    msk = nc.dram_tensor("msk", (128, 128), F32, kind="ExternalInput").ap()
    y = nc.dram_tensor("y", (L, E), F32, kind="ExternalOutput").ap()

    with tile.TileContext(nc) as tc:
        with ExitStack() as stk:
            ec = stk.enter_context
            cpool = ec(tc.tile_pool(name="const", bufs=1))
            ktpool = ec(tc.tile_pool(name="kt", bufs=4))
            vpool = ec(tc.tile_pool(name="vp", bufs=16))
            wkpool = ec(tc.tile_pool(name="wtk", bufs=8))
            wvpool = ec(tc.tile_pool(name="wtv", bufs=8))
            wqpool = ec(tc.tile_pool(name="wtq", bufs=8))
            wopool = ec(tc.tile_pool(name="wo", bufs=4))
            xpool = ec(tc.tile_pool(name="xs", bufs=2))
            qjpool = ec(tc.tile_pool(name="qj", bufs=8))
            a2pool = ec(tc.tile_pool(name="a2", bufs=16))
            ppool = ec(tc.tile_pool(name="pp", bufs=5))
            rpool = ec(tc.tile_pool(name="rp", bufs=2))
            rbpool = ec(tc.tile_pool(name="rb", bufs=2))
            nmpool = ec(tc.tile_pool(name="nm", bufs=2))
            otpool = ec(tc.tile_pool(name="ot", bufs=3))
            sp = ec(tc.tile_pool(name="sp", bufs=2, space="PSUM"))
            op = ec(tc.tile_pool(name="op", bufs=3, space="PSUM"))
            fp = ec(tc.tile_pool(name="fp", bufs=1, space="PSUM"))

            # ---------------- DMA issue: weights & consts ----------------
            # sync queue: wk, xk0, wv, xv0, xq0  (critical path to first MMs)
            # gpsimd queue: small consts, wq, wo (parallel queue)
            bq_sb = cpool.tile([128, 4], F32)
            bk_sb = cpool.tile([128, 4], F32)
            bv_sb = cpool.tile([1, ES], F32R)
            bo_sb = cpool.tile([1, E], F32R)
            mtri_f = cpool.tile([128, 128], F32)
            nc.gpsimd.dma_start(out=bq_sb, in_=bq)
            nc.gpsimd.dma_start(out=bk_sb, in_=bk)
            nc.gpsimd.dma_start(out=bv_sb, in_=bv.bitcast(F32R))
            nc.gpsimd.dma_start(out=bo_sb, in_=bo.bitcast(F32R))
            nc.gpsimd.dma_start(out=mtri_f, in_=msk)

            def load_w(pool, w_dram, nm, eng):
                ts = []
                for ko in range(8):
                    t = pool.tile([128, ES], BF16, tag=f"w{nm}",
                                  name=f"w{nm}{ko}")
                    eng.dma_start(
                        out=t,
                        in_=w_dram[ko * 128:(ko + 1) * 128, :])
                    ts.append(t)
                return ts

            wk_t = load_w(wkpool, wk, "k", nc.sync)
            wv_t = load_w(wvpool, wv, "v", nc.gpsimd)
            wq_t = load_w(wqpool, wq, "q", nc.gpsimd)
            wo_t = []
            for pr in range(4):
                t = wopool.tile([128, E], BF16, tag="wo", name=f"wo{pr}")
                nc.gpsimd.dma_start(
                    out=t, in_=wo[pr * 128:(pr + 1) * 128, :])
                wo_t.append(t)

            # ---------------- persistent on-chip tensors -----------------
            kt = [ktpool.tile([128, L], BF16, tag="kt", name=f"kt{i}")
                  for i in range(4)]
            vts = [vpool.tile([128, 8, 66], BF16, tag="v", name=f"v{i}")
                   for i in range(16)]

            ones_st = cpool.tile([1, 128], F32)
            nc.vector.memset(ones_st, 1.0)
            ones = cpool.tile([1, 128], F32R)
            nc.vector.tensor_copy(ones, ones_st)
            for lv in range(16):
                nc.vector.memset(vts[lv][:, :, 64], 1.0)
            mtri = cpool.tile([128, 2, 128], BF16)
            nc.vector.tensor_copy(mtri[:, 0, :], mtri_f)
            nc.vector.tensor_copy(mtri[:, 1, :], mtri_f)

            # broadcast bias tiles (via ones-matmul, one-time)
            bv_bc = cpool.tile([128, ES], F32)
            bo_bc = cpool.tile([128, E], F32)

            def make_bcasts():
                ps = fp.tile([128, 512], F32, tag="fp", name="psb0")
                nc.tensor.matmul(ps, ones, bv_sb, start=True, stop=True)
                nc.vector.tensor_copy(bv_bc, ps)
                for half in range(2):
                    ps2 = fp.tile([128, 512], F32, tag="fp", name="psb1")
                    nc.tensor.matmul(ps2, ones,
                                     bo_sb[:, half * 512:(half + 1) * 512],
                                     start=True, stop=True)
                    nc.vector.tensor_copy(
                        bo_bc[:, half * 512:(half + 1) * 512], ps2)

            # ---------------- projection chunk emitters ------------------
            x_cache = {}

            def x_tile(x_dram, lb, key, eng=None):
                # NOTE: xpool ring has bufs=2; allocation order must ensure
                # the slot being reused already has its readers emitted.
                if (key, lb) in x_cache:
                    return x_cache[(key, lb)]
                t = xpool.tile([128, 8, 512], BF16, tag="x", name="xt")
                (eng or nc.sync).dma_start(
                    out=t,
                    in_=x_dram.rearrange("(ko ki) l -> ki ko l", ki=128)
                    [:, :, lb * 512:(lb + 1) * 512])
                x_cache[(key, lb)] = t
                return t

            def kproj_chunk(lb, ep):
                # kt[eo][:, lb*512:+512] for eo in {2ep, 2ep+1}
                def emit():
                    xt = x_tile(xk, lb, "k")
                    ps = sp.tile([128, 2, 512], F32, tag="sp", name="psk")
                    for ei in range(2):
                        eo = 2 * ep + ei
                        for ko in range(8):
                            nc.tensor.matmul(
                                ps[:, ei, :],
                                wk_t[ko][:, eo * 128:(eo + 1) * 128],
                                xt[:, ko, :],
                                start=(ko == 0), stop=(ko == 7))
                    for ei in range(2):
                        eo = 2 * ep + ei
                        nc.vector.tensor_scalar_add(
                            kt[eo][:, lb * 512:(lb + 1) * 512],
                            ps[:, ei, :], bk_sb[:, eo:eo + 1])
                return emit

            def vproj_chunk(lb, pair):
                # vts[lb*4 + {2pair, 2pair+1}]
                def emit():
                    xt = x_tile(xv, lb, "v")
                    ps = sp.tile([128, 2, 512], F32, tag="sp", name="psv")
                    for ii in range(2):
                        i = 2 * pair + ii
                        for ko in range(8):
                            nc.tensor.matmul(
                                ps[:, ii, :],
                                xt[:, ko, i * 128:(i + 1) * 128],
                                wv_t[ko], start=(ko == 0), stop=(ko == 7))
                    for ii in range(2):
                        i = 2 * pair + ii
                        lv = lb * 4 + i
                        nc.vector.tensor_add(
                            vts[lv][:, :, 0:64],
                            ps[:, ii, :].rearrange("p (h e) -> p h e", e=64),
                            bv_bc.rearrange("p (h e) -> p h e", e=64))
                return emit

            qj = {}

            def qproj_chunk(jq, ep):
                # qj[(jq, eo)] for eo in {2ep, 2ep+1}
                def emit():
                    xt = x_tile(xq, jq, "q")
                    ps = sp.tile([128, 2, 512], F32, tag="sp", name="psq")
                    for ei in range(2):
                        eo = 2 * ep + ei
                        for ko in range(8):
                            nc.tensor.matmul(
                                ps[:, ei, :],
                                wq_t[ko][:, eo * 128:(eo + 1) * 128],
                                xt[:, ko, :],
                                start=(ko == 0), stop=(ko == 7))
                    for ei in range(2):
                        eo = 2 * ep + ei
                        t = qjpool.tile([128, 512], BF16, tag="qj",
                                        name=f"qj{jq}_{eo}")
                        nc.vector.tensor_scalar_add(
                            t, ps[:, ei, :], bq_sb[:, eo:eo + 1])
                        qj[(jq, eo)] = t
                return emit

            at2 = {}

            def outproj_part(jq, lc, no):
                def emit():
                    a = at2[jq]
                    psf = fp.tile([128, 512], F32, tag="fp", name="psf")
                    for pr in range(4):
                        nc.tensor.matmul(
                            psf,
                            a[pr][:, lc * 128:(lc + 1) * 128],
                            wo_t[pr][:, no * 512:(no + 1) * 512],
                            start=(pr == 0), stop=(pr == 3))
                    ot = otpool.tile([128, 512], F32, tag="ot", name="ot")
                    nc.vector.tensor_add(
                        ot, psf, bo_bc[:, no * 512:(no + 1) * 512])
                    nc.sync.dma_start(
                        out=y[jq * 512 + lc * 128:jq * 512 + (lc + 1) * 128,
                              no * 512:(no + 1) * 512],
                        in_=ot)
                return emit

            # ---------------- attention unit machinery -------------------
            pending = deque()

            def flush_pending(depth):
                while len(pending) > depth:
                    pending.popleft()()

            def mk_av_pair(psoA, psoB, t, kb, n0, nkb, p2):
                def emit():
                    nc.tensor.matmul(
                        psoA[:, n0:512], vts[kb][:, 2 * t, 0:65],
                        p2[:, 0, n0:512],
                        start=(kb == 0), stop=(kb == nkb - 1))
                    nc.tensor.matmul(
                        psoB[:, n0:512], vts[kb][:, 2 * t + 1, 0:65],
                        p2[:, 1, n0:512],
                        start=(kb == 0), stop=(kb == nkb - 1))
                return emit

            def mk_norm(psoA, psoB, t, jq):
                def emit():
                    # reciprocal of denominators (row 64), then broadcast
                    # via K=1 matmul, then normalize into at2 tiles.
                    # reciprocal_approx_fast mis-reads large values straight
                    # from PSUM (negative garbage); stage through SBUF first
                    dsbA = rpool.tile([1, 512], F32, tag="dsb", name="dsbA")
                    nc.vector.tensor_copy(dsbA, psoA[64:65, :])
                    rcf = rpool.tile([1, 512], F32, tag="rcf", name="rcf")
                    nc.vector.reciprocal_approx_fast(rcf, dsbA)
                    rcA = rpool.tile([1, 512], F32R, tag="rc", name="rcA")
                    nc.vector.tensor_copy(rcA, rcf)
                    dsbB = rpool.tile([1, 512], F32, tag="dsb", name="dsbB")
                    nc.vector.tensor_copy(dsbB, psoB[64:65, :])
                    rcg = rpool.tile([1, 512], F32, tag="rcf", name="rcg")
                    nc.vector.reciprocal_approx_fast(rcg, dsbB)
                    rcB = rpool.tile([1, 512], F32R, tag="rc", name="rcB")
                    nc.vector.tensor_copy(rcB, rcg)
                    psbA = fp.tile([64, 512], F32, tag="fp", name="psbA")
                    nc.tensor.matmul(psbA, ones[:, 0:64], rcA,
                                     start=True, stop=True)
                    psbB = fp.tile([64, 512], F32, tag="fp", name="psbB")
                    nc.tensor.matmul(psbB, ones[:, 0:64], rcB,
                                     start=True, stop=True)
                    rbA = rbpool.tile([64, 512], F32R, tag="rb", name="rbA")
                    nc.vector.tensor_copy(rbA, psbA)
                    rbB = rbpool.tile([64, 512], F32R, tag="rb", name="rbB")
                    nc.vector.tensor_copy(rbB, psbB)
                    nc.vector.tensor_mul(at2[jq][t][0:64, :],
                                         psoA[0:64, :], rbA)
                    nrm = nmpool.tile([64, 512], BF16, tag="nrm", name="nrm")
                    nc.vector.tensor_mul(nrm, psoB[0:64, :], rbB)
                    # partition shift 0-63 -> 64-127 via sbuf-sbuf DMA
                    nc.sync.dma_start(out=at2[jq][t][64:128, :], in_=nrm)
                return emit

            def unit(jq, t, kb):
                nkb = 4 * (jq + 1)
                m = kb - 4 * jq
                n0 = 128 * m if m >= 0 else 0
                pss = sp.tile([128, 2, 512], F32, tag="sp", name="pss")
                nc.tensor.matmul(
                    pss[:, 0, n0:512],
                    kt[t][0:64, kb * 128:(kb + 1) * 128],
                    qj[(jq, t)][0:64, n0:512], start=True, stop=True)
                nc.tensor.matmul(
                    pss[:, 1, n0:512],
                    kt[t][64:128, kb * 128:(kb + 1) * 128],
                    qj[(jq, t)][64:128, n0:512], start=True, stop=True)
                flush_pending(4)
                p2 = ppool.tile([128, 2, 512], BF16, tag="p", name="p2")
                nc.scalar.activation(p2[:, :, n0:512], pss[:, :, n0:512],
                                     AF.Exp, scale=float(SCALE))
                if m >= 0:
                    # all-SBUF operands: run on the idle gpsimd engine
                    nc.gpsimd.tensor_mul(
                        p2[:, :, n0:n0 + 128], p2[:, :, n0:n0 + 128], mtri)
                return p2

            # ---------------- opening (phase A head) ---------------------
            # x ring (bufs=2) allocation order: xk0(A), xv0(B), xq0(A after
            # kproj emitted), then per-seg xk/xv prefetch + mid-seg xq kick.
            x_tile(xk, 0, "k")
            x_tile(xv, 0, "v")
            kproj_chunk(0, 0)()
            make_bcasts()
            kproj_chunk(0, 1)()
            x_tile(xq, 0, "q")
            vproj_chunk(0, 0)()
            vproj_chunk(0, 1)()
            qproj_chunk(0, 0)()
            qproj_chunk(0, 1)()

            # ---------------- filler schedules per segment ---------------
            def xq_kick(jq):
                # issue next xq DMA once kproj fillers (readers of the x
                # ring slot being reused) have been emitted
                def emit():
                    x_tile(xq, jq, "q")
                return emit

            # ---------------- main attention stream ----------------------
            # Run-level global schedule. jq=2 and jq=3 runs interleave so
            # the exp-heavy tail shares PE filler (outproj parts) instead
            # of starving once projection work is exhausted.
            for jq in range(4):
                at2[jq] = [a2pool.tile([128, 512], BF16, tag="a2",
                                       name=f"a2_{jq}_{i}")
                           for i in range(4)]

            def pre(lb):
                def em():
                    x_tile(xk, lb, "k")
                    x_tile(xv, lb, "v")
                return em

            op0 = [outproj_part(0, lc, no) for lc in range(4)
                   for no in range(2)]
            op1 = [outproj_part(1, lc, no) for lc in range(4)
                   for no in range(2)]
            op2 = [outproj_part(2, lc, no) for lc in range(4)
                   for no in range(2)]
            op3 = [outproj_part(3, lc, no) for lc in range(4)
                   for no in range(2)]

            schedule = [
                (0, 0, [pre(1), kproj_chunk(1, 0)]),
                (0, 1, [kproj_chunk(1, 1), xq_kick(1)]),
                (0, 2, [vproj_chunk(1, 0), vproj_chunk(1, 1)]),
                (0, 3, [qproj_chunk(1, 0), qproj_chunk(1, 1)]),
                (1, 0, [pre(2), kproj_chunk(2, 0), kproj_chunk(2, 1)]),
                (1, 1, [xq_kick(2), vproj_chunk(2, 0),
                        vproj_chunk(2, 1)]),
                (1, 2, op0[0:4]),
                (1, 3, op0[4:8] + [qproj_chunk(2, 0), qproj_chunk(2, 1)]),
                (2, 0, [pre(3), kproj_chunk(3, 0), kproj_chunk(3, 1),
                        xq_kick(3), vproj_chunk(3, 0), vproj_chunk(3, 1),
                        qproj_chunk(3, 0), qproj_chunk(3, 1)]),
                (3, 0, op1[0:3]),
                (2, 1, op1[3:6]),
                (3, 1, op1[6:8]),
                (2, 2, []),
                (3, 2, []),
                (2, 3, []),
                (3, 3, op2[0:8]),
            ]
            seen_jq = set()
            for jq, t, fills in schedule:
                if jq not in seen_jq:
                    # force pending (incl. this jq's qproj fillers) to have
                    # emitted before the first unit references qj[(jq, t)]
                    flush_pending(0)
                    seen_jq.add(jq)
                nkb = 4 * (jq + 1)
                psoA = op.tile([65, 512], F32, tag="op", name="psoA")
                psoB = op.tile([65, 512], F32, tag="op", name="psoB")
                done_fill = 0
                for kb in range(nkb):
                    p2 = unit(jq, t, kb)
                    m = kb - 4 * jq
                    n0 = 128 * m if m >= 0 else 0
                    pending.append(
                        mk_av_pair(psoA, psoB, t, kb, n0, nkb, p2))
                    want = ((kb + 1) * len(fills)) // nkb
                    while done_fill < want:
                        pending.append(fills[done_fill])
                        done_fill += 1
                pending.append(mk_norm(psoA, psoB, t, jq))
            def outproj_tail_pair(pairi):
                # final out-projection: sp pool (idle at the tail) provides
                # two psum banks per pair so parts pipeline instead of
                # serializing on the single fp bank
                def emit():
                    a = at2[3]
                    psf2 = sp.tile([128, 2, 512], F32, tag="sp",
                                   name="psf2")
                    for half in range(2):
                        lc, no = pairi, half
                        for pr in range(4):
                            nc.tensor.matmul(
                                psf2[:, half, :],
                                a[pr][:, lc * 128:(lc + 1) * 128],
                                wo_t[pr][:, no * 512:(no + 1) * 512],
                                start=(pr == 0), stop=(pr == 3))
                    for half in range(2):
                        lc, no = pairi, half
                        ot = otpool.tile([128, 512], F32, tag="ot",
                                         name="ot")
                        nc.vector.tensor_add(
                            ot, psf2[:, half, :],
                            bo_bc[:, no * 512:(no + 1) * 512])
                        nc.sync.dma_start(
                            out=y[3 * 512 + lc * 128:
                                  3 * 512 + (lc + 1) * 128,
                                  no * 512:(no + 1) * 512],
                            in_=ot)
                return emit

            for pairi in range(4):
                pending.append(outproj_tail_pair(pairi))
            flush_pending(0)

    nc.finalize()
    return nc


def _make_tri():
    kk = np.arange(128)[:, None]
    jj = np.arange(128)[None, :]
    return (jj >= kk).astype(np.float32)


def make_in_maps(query, key, value, W_packed, b_packed, W_out, b_out):
    query = np.asarray(query, dtype=np.float32)
    key = np.asarray(key, dtype=np.float32)
    value = np.asarray(value, dtype=np.float32)
    W_packed = np.asarray(W_packed, dtype=np.float32)
    b_packed = np.asarray(b_packed, dtype=np.float32)
    W_out = np.asarray(W_out, dtype=np.float32)
    b_out = np.asarray(b_out, dtype=np.float32)

    msk = _make_tri()
    BF = ml_dtypes.bfloat16
    xqT = [np.ascontiguousarray(query[b].T).astype(BF) for b in range(N)]
    xkT = [np.ascontiguousarray(key[b].T).astype(BF) for b in range(N)]
    xvT = [np.ascontiguousarray(value[b].T).astype(BF) for b in range(N)]

    in_maps = []
    for c in range(NCORES):
        b, g = c // 2, c % 2
        sl = slice(g * ES, (g + 1) * ES)
        in_maps.append({
            "xq": xqT[b], "xk": xkT[b], "xv": xvT[b],
            "wq": np.ascontiguousarray(
                W_packed[0 * E:][:E][sl, :].T).astype(BF),
            "wk": np.ascontiguousarray(
                W_packed[1 * E:][:E][sl, :].T).astype(BF),
            "wv": np.ascontiguousarray(
                W_packed[2 * E:][:E][sl, :].T).astype(BF),
            "wo": np.ascontiguousarray(W_out[:, sl].T).astype(BF),
            "bq": np.ascontiguousarray(
                b_packed[0 * E:][:E][sl].reshape(4, 128).T),
            "bk": np.ascontiguousarray(
                b_packed[1 * E:][:E][sl].reshape(4, 128).T),
            "bv": b_packed[2 * E:][:E][sl].reshape(1, ES).copy(),
            "bo": (b_out.reshape(1, E).copy() if g == 0
                   else np.zeros((1, E), np.float32)),
            "msk": msk,
        })
    return in_maps


def get_nc():
    if "nc" not in _CACHE:
        _CACHE["nc"] = _build()
    return _CACHE["nc"]


def kernel(query, key, value, W_packed, b_packed, W_out, b_out):
    nc = get_nc()
    in_maps = make_in_maps(query, key, value, W_packed, b_packed,
                           W_out, b_out)
    res = bass_utils.run_bass_kernel_spmd(nc, in_maps,
                                          core_ids=list(range(NCORES)))
    out = np.stack([res.results[2 * b]["y"] + res.results[2 * b + 1]["y"]
                    for b in range(N)])
    return out.astype(np.float32)


# revision 31
# speedup vs baseline: 1.6035x; 1.0287x over previous
"""Multi-head causal self-attention (N=4, L=2048, E=1024, H=16) on 8 NeuronCores.

Sharding: core c handles batch b = c//2 and head-group g = c%2 (8 heads,
E-slice of 512). Each core computes its QKV projection slice, causal
attention for its 8 heads, and a partial out-projection (E-contraction over
its 512-slice). Host sums the two partials per batch (bias added on g=0).

Key structure (v2, rewritten from the 533us baseline):
  - Scores for the two heads of a kt/qj tile run as ROW-TILED CONCURRENT
    matmul pairs (K=64 each, rows 0-63 / 64-127) -> one 512-cycle slot
    covers both heads, and each pair's LDWEIGHTS pulls ahead under the
    other's matmul (disjoint row groups).
  - Diagonal 128-blocks stream only columns >= 128*m (rest is causally
    masked): shorter score/AV matmuls and shorter exps.
  - kt/qj/p2/vts are bf16 (tolerance 2e-2 vs measured ~2e-4 error budget);
    exp output is bf16, mask-mul on DVE gets 2x mode, AV matmul is bf16.
  - v tiles carry a ones column (65th) so the AV matmul also produces the
    softmax denominator (row 64 of the [65,512] psum).
  - All bias adds that needed K=1 matmuls now ride on DVE/GPSIMD adds with
    pre-broadcast bias tiles; q/k biases stay on ACT (per-partition bias).
  - Projection work for l-blocks 1..3 and q-blocks 1..3 is deferred and
    interleaved into the attention unit stream as PE filler (deadline
    scheduled) so the PE never starves while ACT runs exp -> HAM stays at
    K=8/8.
"""

from collections import deque
from contextlib import ExitStack

import ml_dtypes
import numpy as np

import concourse.bacc as bacc
import concourse.mybir as mybir
import concourse.tile as tile
from concourse import bass_utils

F32 = mybir.dt.float32
F32R = mybir.dt.float32r
BF16 = mybir.dt.bfloat16
AF = mybir.ActivationFunctionType

N, L, E = 4, 2048, 1024
H, EH = 16, 64
NCORES = 8
ES = 512          # e-slice per core (8 heads x 64)
SCALE = 1.0 / np.sqrt(EH)

_CACHE = {}


def _build():
    nc = bacc.Bacc("TRN2", target_bir_lowering=False, debug=False,
                   num_devices=NCORES)
    xq = nc.dram_tensor("xq", (E, L), BF16, kind="ExternalInput").ap()
    xk = nc.dram_tensor("xk", (E, L), BF16, kind="ExternalInput").ap()
    xv = nc.dram_tensor("xv", (E, L), BF16, kind="ExternalInput").ap()
    wq = nc.dram_tensor("wq", (E, ES), BF16, kind="ExternalInput").ap()
    wk = nc.dram_tensor("wk", (E, ES), BF16, kind="ExternalInput").ap()
    wv = nc.dram_tensor("wv", (E, ES), BF16, kind="ExternalInput").ap()
    wo = nc.dram_tensor("wo", (ES, E), BF16, kind="ExternalInput").ap()
    bq = nc.dram_tensor("bq", (128, 4), F32, kind="ExternalInput").ap()
    bk = nc.dram_tensor("bk", (128, 4), F32, kind="ExternalInput").ap()
    bv = nc.dram_tensor("bv", (1, ES), F32, kind="ExternalInput").ap()
    bo = nc.dram_tensor("bo", (1, E), F32, kind="ExternalInput").ap()
    msk = nc.dram_tensor("msk", (128, 128), F32, kind="ExternalInput").ap()
    y = nc.dram_tensor("y", (L, E), F32, kind="ExternalOutput").ap()

    with tile.TileContext(nc) as tc:
        with ExitStack() as stk:
            ec = stk.enter_context
            cpool = ec(tc.tile_pool(name="const", bufs=1))
            ktpool = ec(tc.tile_pool(name="kt", bufs=4))
            vpool = ec(tc.tile_pool(name="vp", bufs=16))
            wkpool = ec(tc.tile_pool(name="wtk", bufs=8))
            wvpool = ec(tc.tile_pool(name="wtv", bufs=8))
            wqpool = ec(tc.tile_pool(name="wtq", bufs=8))
            wopool = ec(tc.tile_pool(name="wo", bufs=4))
            xpool = ec(tc.tile_pool(name="xs", bufs=2))
            qjpool = ec(tc.tile_pool(name="qj", bufs=8))
            a2pool = ec(tc.tile_pool(name="a2", bufs=16))
            ppool = ec(tc.tile_pool(name="pp", bufs=6))
            rpool = ec(tc.tile_pool(name="rp", bufs=2))
            rbpool = ec(tc.tile_pool(name="rb", bufs=2))
            nmpool = ec(tc.tile_pool(name="nm", bufs=2))
            otpool = ec(tc.tile_pool(name="ot", bufs=3))
            sp = ec(tc.tile_pool(name="sp", bufs=2, space="PSUM"))
            op = ec(tc.tile_pool(name="op", bufs=3, space="PSUM"))
            fp = ec(tc.tile_pool(name="fp", bufs=1, space="PSUM"))

            # ---------------- DMA issue: weights & consts ----------------
            # sync queue: wk, xk0, wv, xv0, xq0  (critical path to first MMs)
            # gpsimd queue: small consts, wq, wo (parallel queue)
            bq_sb = cpool.tile([128, 4], F32)
            bk_sb = cpool.tile([128, 4], F32)
            bv_sb = cpool.tile([1, ES], F32R)
            bo_sb = cpool.tile([1, E], F32R)
            mtri_f = cpool.tile([128, 128], F32)
            nc.gpsimd.dma_start(out=bq_sb, in_=bq)
            nc.gpsimd.dma_start(out=bk_sb, in_=bk)
            nc.gpsimd.dma_start(out=bv_sb, in_=bv.bitcast(F32R))
            nc.gpsimd.dma_start(out=bo_sb, in_=bo.bitcast(F32R))
            nc.gpsimd.dma_start(out=mtri_f, in_=msk)

            def load_w(pool, w_dram, nm, eng, split=False):
                ts = []
                for ko in range(8):
                    t = pool.tile([128, ES], BF16, tag=f"w{nm}",
                                  name=f"w{nm}{ko}")
                    if split and ko % 2 == 1:
                        eng = nc.gpsimd
                    elif split:
                        eng = nc.sync
                    eng.dma_start(
                        out=t,
                        in_=w_dram[ko * 128:(ko + 1) * 128, :])
                    ts.append(t)
                return ts

            wk_t = load_w(wkpool, wk, "k", nc.sync, split=True)
            wv_t = load_w(wvpool, wv, "v", nc.gpsimd)
            wq_t = load_w(wqpool, wq, "q", nc.gpsimd)
            wo_t = []
            for pr in range(4):
                t = wopool.tile([128, E], BF16, tag="wo", name=f"wo{pr}")
                nc.gpsimd.dma_start(
                    out=t, in_=wo[pr * 128:(pr + 1) * 128, :])
                wo_t.append(t)

            # ---------------- persistent on-chip tensors -----------------
            kt = [ktpool.tile([128, L], BF16, tag="kt", name=f"kt{i}")
                  for i in range(4)]
            vts = [vpool.tile([128, 8, 66], BF16, tag="v", name=f"v{i}")
                   for i in range(16)]

            ones_st = cpool.tile([1, 128], F32)
            nc.vector.memset(ones_st, 1.0)
            ones = cpool.tile([1, 128], F32R)
            nc.vector.tensor_copy(ones, ones_st)
            for lv in range(16):
                nc.vector.memset(vts[lv][:, :, 64], 1.0)
            mtri = cpool.tile([128, 2, 128], BF16)
            nc.vector.tensor_copy(mtri[:, 0, :], mtri_f)
            nc.vector.tensor_copy(mtri[:, 1, :], mtri_f)

            # broadcast bias tiles (via ones-matmul, one-time)
            bv_bc = cpool.tile([128, ES], F32)
            bo_bc = cpool.tile([128, E], F32)

            def make_bcasts():
                ps = fp.tile([128, 512], F32, tag="fp", name="psb0")
                nc.tensor.matmul(ps, ones, bv_sb, start=True, stop=True)
                nc.vector.tensor_copy(bv_bc, ps)
                for half in range(2):
                    ps2 = fp.tile([128, 512], F32, tag="fp", name="psb1")
                    nc.tensor.matmul(ps2, ones,
                                     bo_sb[:, half * 512:(half + 1) * 512],
                                     start=True, stop=True)
                    nc.vector.tensor_copy(
                        bo_bc[:, half * 512:(half + 1) * 512], ps2)

            # ---------------- projection chunk emitters ------------------
            x_cache = {}

            def x_tile(x_dram, lb, key, eng=None):
                # NOTE: xpool ring has bufs=2; allocation order must ensure
                # the slot being reused already has its readers emitted.
                if (key, lb) in x_cache:
                    return x_cache[(key, lb)]
                t = xpool.tile([128, 8, 512], BF16, tag="x", name="xt")
                (eng or nc.sync).dma_start(
                    out=t,
                    in_=x_dram.rearrange("(ko ki) l -> ki ko l", ki=128)
                    [:, :, lb * 512:(lb + 1) * 512])
                x_cache[(key, lb)] = t
                return t

            def kproj_chunk(lb, ep):
                # kt[eo][:, lb*512:+512] for eo in {2ep, 2ep+1}
                def emit():
                    xt = x_tile(xk, lb, "k")
                    ps = sp.tile([128, 2, 512], F32, tag="sp", name="psk")
                    for ei in range(2):
                        eo = 2 * ep + ei
                        for ko in range(8):
                            nc.tensor.matmul(
                                ps[:, ei, :],
                                wk_t[ko][:, eo * 128:(eo + 1) * 128],
                                xt[:, ko, :],
                                start=(ko == 0), stop=(ko == 7))
                    for ei in range(2):
                        eo = 2 * ep + ei
                        nc.vector.tensor_scalar_add(
                            kt[eo][:, lb * 512:(lb + 1) * 512],
                            ps[:, ei, :], bk_sb[:, eo:eo + 1])
                return emit

            def vproj_chunk(lb, pair):
                # vts[lb*4 + {2pair, 2pair+1}]
                def emit():
                    xt = x_tile(xv, lb, "v")
                    ps = sp.tile([128, 2, 512], F32, tag="sp", name="psv")
                    for ii in range(2):
                        i = 2 * pair + ii
                        for ko in range(8):
                            nc.tensor.matmul(
                                ps[:, ii, :],
                                xt[:, ko, i * 128:(i + 1) * 128],
                                wv_t[ko], start=(ko == 0), stop=(ko == 7))
                    for ii in range(2):
                        i = 2 * pair + ii
                        lv = lb * 4 + i
                        nc.vector.tensor_add(
                            vts[lv][:, :, 0:64],
                            ps[:, ii, :].rearrange("p (h e) -> p h e", e=64),
                            bv_bc.rearrange("p (h e) -> p h e", e=64))
                return emit

            qj = {}

            def qproj_chunk(jq, ep):
                # qj[(jq, eo)] for eo in {2ep, 2ep+1}
                def emit():
                    xt = x_tile(xq, jq, "q")
                    ps = sp.tile([128, 2, 512], F32, tag="sp", name="psq")
                    for ei in range(2):
                        eo = 2 * ep + ei
                        for ko in range(8):
                            nc.tensor.matmul(
                                ps[:, ei, :],
                                wq_t[ko][:, eo * 128:(eo + 1) * 128],
                                xt[:, ko, :],
                                start=(ko == 0), stop=(ko == 7))
                    for ei in range(2):
                        eo = 2 * ep + ei
                        t = qjpool.tile([128, 512], BF16, tag="qj",
                                        name=f"qj{jq}_{eo}")
                        nc.vector.tensor_scalar_add(
                            t, ps[:, ei, :], bq_sb[:, eo:eo + 1])
                        qj[(jq, eo)] = t
                return emit

            at2 = {}

            def outproj_part(jq, lc, no):
                def emit():
                    a = at2[jq]
                    psf = fp.tile([128, 512], F32, tag="fp", name="psf")
                    for pr in range(4):
                        nc.tensor.matmul(
                            psf,
                            a[pr][:, lc * 128:(lc + 1) * 128],
                            wo_t[pr][:, no * 512:(no + 1) * 512],
                            start=(pr == 0), stop=(pr == 3))
                    ot = otpool.tile([128, 512], F32, tag="ot", name="ot")
                    nc.vector.tensor_add(
                        ot, psf, bo_bc[:, no * 512:(no + 1) * 512])
                    nc.sync.dma_start(
                        out=y[jq * 512 + lc * 128:jq * 512 + (lc + 1) * 128,
                              no * 512:(no + 1) * 512],
                        in_=ot)
                return emit

            # ---------------- attention unit machinery -------------------
            pending = deque()

            def flush_pending(depth):
                while len(pending) > depth:
                    pending.popleft()()

            def mk_av_pair(psoA, psoB, t, kb, n0, nkb, p2):
                def emit():
                    nc.tensor.matmul(
                        psoA[:, n0:512], vts[kb][:, 2 * t, 0:65],
                        p2[:, 0, n0:512],
                        start=(kb == 0), stop=(kb == nkb - 1))
                    nc.tensor.matmul(
                        psoB[:, n0:512], vts[kb][:, 2 * t + 1, 0:65],
                        p2[:, 1, n0:512],
                        start=(kb == 0), stop=(kb == nkb - 1))
                return emit

            def mk_norm(psoA, psoB, t, jq):
                def emit():
                    # reciprocal of denominators (row 64), then broadcast
                    # via K=1 matmul, then normalize into at2 tiles.
                    # reciprocal_approx_fast mis-reads large values straight
                    # from PSUM (negative garbage); stage through SBUF first
                    dsbA = rpool.tile([1, 512], F32, tag="dsb", name="dsbA")
                    nc.vector.tensor_copy(dsbA, psoA[64:65, :])
                    rcf = rpool.tile([1, 512], F32, tag="rcf", name="rcf")
                    nc.vector.reciprocal_approx_fast(rcf, dsbA)
                    rcA = rpool.tile([1, 512], F32R, tag="rc", name="rcA")
                    nc.vector.tensor_copy(rcA, rcf)
                    dsbB = rpool.tile([1, 512], F32, tag="dsb", name="dsbB")
                    nc.vector.tensor_copy(dsbB, psoB[64:65, :])
                    rcg = rpool.tile([1, 512], F32, tag="rcf", name="rcg")
                    nc.vector.reciprocal_approx_fast(rcg, dsbB)
                    rcB = rpool.tile([1, 512], F32R, tag="rc", name="rcB")
                    nc.vector.tensor_copy(rcB, rcg)
                    psbA = fp.tile([64, 512], F32, tag="fp", name="psbA")
                    nc.tensor.matmul(psbA, ones[:, 0:64], rcA,
                                     start=True, stop=True)
                    psbB = fp.tile([64, 512], F32, tag="fp", name="psbB")
                    nc.tensor.matmul(psbB, ones[:, 0:64], rcB,
                                     start=True, stop=True)
                    rbA = rbpool.tile([64, 512], F32R, tag="rb", name="rbA")
                    nc.vector.tensor_copy(rbA, psbA)
                    rbB = rbpool.tile([64, 512], F32R, tag="rb", name="rbB")
                    nc.vector.tensor_copy(rbB, psbB)
                    nc.vector.tensor_mul(at2[jq][t][0:64, :],
                                         psoA[0:64, :], rbA)
                    nrm = nmpool.tile([64, 512], BF16, tag="nrm", name="nrm")
                    nc.vector.tensor_mul(nrm, psoB[0:64, :], rbB)
                    # partition shift 0-63 -> 64-127 via sbuf-sbuf DMA
                    nc.sync.dma_start(out=at2[jq][t][64:128, :], in_=nrm)
                return emit

            def unit(jq, t, kb):
                nkb = 4 * (jq + 1)
                m = kb - 4 * jq
                n0 = 128 * m if m >= 0 else 0
                pss = sp.tile([128, 2, 512], F32, tag="sp", name="pss")
                nc.tensor.matmul(
                    pss[:, 0, n0:512],
                    kt[t][0:64, kb * 128:(kb + 1) * 128],
                    qj[(jq, t)][0:64, n0:512], start=True, stop=True)
                nc.tensor.matmul(
                    pss[:, 1, n0:512],
                    kt[t][64:128, kb * 128:(kb + 1) * 128],
                    qj[(jq, t)][64:128, n0:512], start=True, stop=True)
                flush_pending(5)
                p2 = ppool.tile([128, 2, 512], BF16, tag="p", name="p2")
                nc.scalar.activation(p2[:, :, n0:512], pss[:, :, n0:512],
                                     AF.Exp, scale=float(SCALE))
                if m >= 0:
                    # all-SBUF operands: run on the idle gpsimd engine
                    nc.gpsimd.tensor_mul(
                        p2[:, :, n0:n0 + 128], p2[:, :, n0:n0 + 128], mtri)
                return p2

            # ---------------- opening (phase A head) ---------------------
            # x ring (bufs=2) allocation order: xk0(A), xv0(B), xq0(A after
            # kproj emitted), then per-seg xk/xv prefetch + mid-seg xq kick.
            x_tile(xk, 0, "k")
            x_tile(xv, 0, "v")
            kproj_chunk(0, 0)()
            make_bcasts()
            kproj_chunk(0, 1)()
            x_tile(xq, 0, "q")
            vproj_chunk(0, 0)()
            vproj_chunk(0, 1)()
            qproj_chunk(0, 0)()

            # ---------------- filler schedules per segment ---------------
            def xq_kick(jq):
                # issue next xq DMA once kproj fillers (readers of the x
                # ring slot being reused) have been emitted
                def emit():
                    x_tile(xq, jq, "q")
                return emit

            # ---------------- main attention stream ----------------------
            # Run-level global schedule. jq=2 and jq=3 runs interleave so
            # the exp-heavy tail shares PE filler (outproj parts) instead
            # of starving once projection work is exhausted.
            for jq in range(4):
                at2[jq] = [a2pool.tile([128, 512], BF16, tag="a2",
                                       name=f"a2_{jq}_{i}")
                           for i in range(4)]

            def pre(lb):
                def em():
                    x_tile(xk, lb, "k")
                    x_tile(xv, lb, "v")
                return em

            op0 = [outproj_part(0, lc, no) for lc in range(4)
                   for no in range(2)]
            op1 = [outproj_part(1, lc, no) for lc in range(4)
                   for no in range(2)]
            op2 = [outproj_part(2, lc, no) for lc in range(4)
                   for no in range(2)]
            op3 = [outproj_part(3, lc, no) for lc in range(4)
                   for no in range(2)]

            schedule = [
                (0, 0, [qproj_chunk(0, 1), pre(1), kproj_chunk(1, 0)]),
                (0, 1, [kproj_chunk(1, 1), xq_kick(1)]),
                (0, 2, [vproj_chunk(1, 0), vproj_chunk(1, 1)]),
                (0, 3, [qproj_chunk(1, 0), qproj_chunk(1, 1)]),
                (1, 0, [pre(2), kproj_chunk(2, 0), kproj_chunk(2, 1)]),
                (1, 1, [xq_kick(2), vproj_chunk(2, 0),
                        vproj_chunk(2, 1)]),
                (1, 2, op0[0:4]),
                (1, 3, op0[4:8] + [qproj_chunk(2, 0), qproj_chunk(2, 1)]),
                (2, 0, [pre(3), kproj_chunk(3, 0), kproj_chunk(3, 1),
                        xq_kick(3), vproj_chunk(3, 0), vproj_chunk(3, 1),
                        qproj_chunk(3, 0), qproj_chunk(3, 1)]),
                (3, 0, op1[0:3]),
                (2, 1, op1[3:6]),
                (3, 1, op1[6:8]),
                (2, 2, []),
                (3, 2, []),
                (2, 3, []),
                (3, 3, op2[0:8]),
            ]
            seen_jq = set()
            for jq, t, fills in schedule:
                if jq not in seen_jq:
                    # force pending (incl. this jq's qproj fillers) to have
                    # emitted before the first unit references qj[(jq, t)]
                    flush_pending(0)
                    seen_jq.add(jq)
                nkb = 4 * (jq + 1)
                psoA = op.tile([65, 512], F32, tag="op", name="psoA")
                psoB = op.tile([65, 512], F32, tag="op", name="psoB")
                done_fill = 0
                for kb in range(nkb):
                    p2 = unit(jq, t, kb)
                    m = kb - 4 * jq
                    n0 = 128 * m if m >= 0 else 0
                    pending.append(
                        mk_av_pair(psoA, psoB, t, kb, n0, nkb, p2))
                    want = ((kb + 1) * len(fills)) // nkb
                    while done_fill < want:
                        pending.append(fills[done_fill])
                        done_fill += 1
                pending.append(mk_norm(psoA, psoB, t, jq))
            def outproj_tail_pair(pairi):
                # final out-projection: sp pool (idle at the tail) provides
                # two psum banks per pair so parts pipeline instead of
                # serializing on the single fp bank
                def emit():
                    a = at2[3]
                    psf2 = sp.tile([128, 2, 512], F32, tag="sp",
                                   name="psf2")
                    for half in range(2):
                        lc, no = pairi, half
                        for pr in range(4):
                            nc.tensor.matmul(
                                psf2[:, half, :],
                                a[pr][:, lc * 128:(lc + 1) * 128],
                                wo_t[pr][:, no * 512:(no + 1) * 512],
                                start=(pr == 0), stop=(pr == 3))
                    for half in range(2):
                        lc, no = pairi, half
                        ot = otpool.tile([128, 512], F32, tag="ot",
                                         name="ot")
                        nc.vector.tensor_add(
                            ot, psf2[:, half, :],
                            bo_bc[:, no * 512:(no + 1) * 512])
                        nc.sync.dma_start(
                            out=y[3 * 512 + lc * 128:
                                  3 * 512 + (lc + 1) * 128,
                                  no * 512:(no + 1) * 512],
                            in_=ot)
                return emit

            for pairi in range(4):
                pending.append(outproj_tail_pair(pairi))
            flush_pending(0)

    nc.finalize()
    return nc


def _make_tri():
    kk = np.arange(128)[:, None]
    jj = np.arange(128)[None, :]
    return (jj >= kk).astype(np.float32)


def make_in_maps(query, key, value, W_packed, b_packed, W_out, b_out):
    query = np.asarray(query, dtype=np.float32)
    key = np.asarray(key, dtype=np.float32)
    value = np.asarray(value, dtype=np.float32)
    W_packed = np.asarray(W_packed, dtype=np.float32)
    b_packed = np.asarray(b_packed, dtype=np.float32)
    W_out = np.asarray(W_out, dtype=np.float32)
    b_out = np.asarray(b_out, dtype=np.float32)

    msk = _make_tri()
    BF = ml_dtypes.bfloat16
    xqT = [np.ascontiguousarray(query[b].T).astype(BF) for b in range(N)]
    xkT = [np.ascontiguousarray(key[b].T).astype(BF) for b in range(N)]
    xvT = [np.ascontiguousarray(value[b].T).astype(BF) for b in range(N)]

    in_maps = []
    for c in range(NCORES):
        b, g = c // 2, c % 2
        sl = slice(g * ES, (g + 1) * ES)
        in_maps.append({
            "xq": xqT[b], "xk": xkT[b], "xv": xvT[b],
            "wq": np.ascontiguousarray(
                W_packed[0 * E:][:E][sl, :].T).astype(BF),
            "wk": np.ascontiguousarray(
                W_packed[1 * E:][:E][sl, :].T).astype(BF),
            "wv": np.ascontiguousarray(
                W_packed[2 * E:][:E][sl, :].T).astype(BF),
            "wo": np.ascontiguousarray(W_out[:, sl].T).astype(BF),
            "bq": np.ascontiguousarray(
                b_packed[0 * E:][:E][sl].reshape(4, 128).T),
            "bk": np.ascontiguousarray(
                b_packed[1 * E:][:E][sl].reshape(4, 128).T),
            "bv": b_packed[2 * E:][:E][sl].reshape(1, ES).copy(),
            "bo": (b_out.reshape(1, E).copy() if g == 0
                   else np.zeros((1, E), np.float32)),
            "msk": msk,
        })
    return in_maps


def get_nc():
    if "nc" not in _CACHE:
        _CACHE["nc"] = _build()
    return _CACHE["nc"]


def kernel(query, key, value, W_packed, b_packed, W_out, b_out):
    nc = get_nc()
    in_maps = make_in_maps(query, key, value, W_packed, b_packed,
                           W_out, b_out)
    res = bass_utils.run_bass_kernel_spmd(nc, in_maps,
                                          core_ids=list(range(NCORES)))
    out = np.stack([res.results[2 * b]["y"] + res.results[2 * b + 1]["y"]
                    for b in range(N)])
    return out.astype(np.float32)
